# revision 24
# baseline (speedup 1.0000x reference)
"""Trainium2 kernel for MagFace/AdaCos-style margin softmax-CE loss.

Strategy (8 cores, class-parallel, fp8 DoubleRow):
  - Shard C=100000 classes across 8 cores (12500 each, zero-padded to
    12544 = 28 chunks of 448).
  - Host pre-normalizes weight rows (w_c/||w_c||, scaled by 64) and ships
    them fp8e4m3 in a transposed [d -> class] layout; x is shipped both
    raw fp32 (for norms + the label-side margin math) and as a 16x-scaled
    fp8 transposed copy (the matmul stationary operand).
  - Per core: matmul orientation is out[128 samples, 448 classes]:
    stationary lhsT = xT[128d, 2, 128b] fp8, moving rhs = wT[128d, 2, 448c]
    fp8 with perf_mode=DoubleRow (256-contraction per instr, 2 instr per
    chunk cover D=512). PSUM tiles [128, 4, 512] (4 banks), double-buffered.
  - ScalarE: one exp activation per (b_block, stripe-of-4-chunks) reading
    [128, 4, 448] from PSUM with per-partition scale S/(1024*||x_b||);
    accum_out yields the per-sample sum-exp directly. This is the pace
    setter (~1.92us per activation, 28 total).
  - Top-1 accuracy needs max_c cos_c only coarsely: the per-stripe sums
    bound it (ln(m)/S >= maxcos >= ln(m/1792)/S); ambiguous samples (none
    for this data, margin ~0.09) fall back to an exact host check.
  - The label-column margin math (phi) runs off the critical path from
    host-gathered label rows W[label].
  - Host combines per-core sums, corrects the label column, adds the
    MagFace regularizer, computes top-1 accuracy.
"""

import math
import sys

sys.path.insert(0, "/opt/trn_rl_repo")
sys.path.insert(0, "/opt/trn_rl_repo/concourse")

import numpy as np

# ---- problem constants ----
B = 512
D = 512
C = 100000
NCORES = 8
C_SH = C // NCORES          # 12500
CHUNK = 448
NCHUNK = 28                 # 28 * 448 = 12544
C_PAD = NCHUNK * CHUNK      # 12544
NPADC = C_PAD - C_SH        # 44 zero-pad classes per core
STRIPE = 4                  # chunks per activation group
NSTRIPE = NCHUNK // STRIPE  # 7
SCLS = STRIPE * CHUNK       # 1792 classes per stripe
S = 30.0
N_U = 110.0
N_L = 10.0
M_U = 1.0
M_L = 0.1
LAMBDA_G = 35.0
XSCALE = 16.0               # host scale folded into fp8 x
WSCALE = 64.0               # host scale folded into fp8 normalized weights

_cache = {}


def _emit_rsqrt(nc, pp_tiles, out, n2_ap, G, final_mul=1.0):
    """out = final_mul / sqrt(n2) via bit-trick seed + 2 Newton iterations."""
    import concourse.mybir as mybir

    ALU = mybir.AluOpType
    magic, sh, yi, h, t1, t2 = pp_tiles
    n2i = n2_ap.bitcast(mybir.dt.int32)
    nc.vector.tensor_scalar(
        out=sh[:, :G], in0=n2i, scalar1=1, scalar2=None,
        op0=ALU.logical_shift_right,
    )
    nc.vector.tensor_sub(yi[:, :G], magic[:, :G], sh[:, :G])
    y = yi[:, :G].bitcast(mybir.dt.float32)
    nc.vector.tensor_scalar(
        out=h[:, :G], in0=n2_ap, scalar1=0.5, scalar2=None, op0=ALU.mult
    )
    # iter 1
    nc.vector.tensor_mul(t1[:, :G], y, y)
    nc.vector.tensor_mul(t1[:, :G], t1[:, :G], h[:, :G])
    nc.vector.tensor_scalar(
        out=t2[:, :G], in0=t1[:, :G], scalar1=-1.0, scalar2=1.5,
        op0=ALU.mult, op1=ALU.add,
    )
    nc.vector.tensor_mul(t2[:, :G], t2[:, :G], y)
    # iter 2 (fold final_mul into the last step)
    nc.vector.tensor_mul(t1[:, :G], t2[:, :G], t2[:, :G])
    nc.vector.tensor_mul(t1[:, :G], t1[:, :G], h[:, :G])
    nc.vector.tensor_scalar(
        out=t1[:, :G], in0=t1[:, :G], scalar1=-final_mul, scalar2=1.5 * final_mul,
        op0=ALU.mult, op1=ALU.add,
    )
    nc.vector.tensor_mul(out, t1[:, :G], t2[:, :G])


def _emit_body(nc, tc, tensors, mybir, bass):
    F32 = mybir.dt.float32
    BF16 = mybir.dt.bfloat16
    FP8 = mybir.dt.float8e4
    I32 = mybir.dt.int32
    ALU = mybir.AluOpType
    ACT = mybir.ActivationFunctionType
    DR = mybir.MatmulPerfMode.DoubleRow

    x_dram = tensors["x"]
    xt_dram = tensors["xt"]
    wt_dram = tensors["wt"]
    wl_dram = tensors["wl"]
    sums_dram = tensors["sums"]
    misc_dram = tensors["misc"]
    wt_ap = wt_dram.ap()

    with (
        tc.tile_pool(name="persist", bufs=1) as pp,
        tc.tile_pool(name="psum", bufs=2, space=bass.MemorySpace.PSUM) as psum_pool,
    ):
        # ---- phase 0a: critical-path loads + exp table preload + PE warmup ----
        warm_src = pp.tile([128, 1], F32)
        nc.gpsimd.memset(warm_src[:], 0.0)
        warm_dst = pp.tile([128, 1], F32)
        nc.scalar.activation(warm_dst[:], warm_src[:], ACT.Exp)

        # PE warmup operands (memset, no DMA dependency): HAM unthrottles
        # during the DMA lead-in so the real matmul stream runs warm.
        warm_w = pp.tile([128, 2, 128], FP8)
        nc.gpsimd.memset(warm_w[:], 0)
        warm_m = pp.tile([128, 2, 512], FP8)
        nc.gpsimd.memset(warm_m[:], 0)
        warm_ps = psum_pool.tile([128, STRIPE, 512], F32, tag="ps")
        for _ in range(16):
            nc.tensor.matmul(
                warm_ps[:, 0, :], warm_w[:], warm_m[:],
                start=True, stop=True, perf_mode=DR,
            )

        # DMA order on the sync queue: x_t0 (gates scale0), xt (stationary),
        # wt stripe 0, then the rest of x interleaved with early stripes.
        x_sb = pp.tile([128, 4, D], BF16)
        x_r = x_dram.ap().rearrange("(t p) d -> p t d", p=128)
        xt_sb = pp.tile([128, 2, 2, B], FP8)
        wt_all = pp.tile([128, NSTRIPE, 2, 2, SCLS], FP8)

        def _wt_load(s):
            if s < 3:
                # early stripes in halves: finer arrival granularity while
                # the PE prefetch is still ahead of the DMA stream
                h = SCLS // 2
                nc.sync.dma_start(wt_all[:, s, :, :, :h], wt_ap[:, s, :, :, :h])
                nc.sync.dma_start(wt_all[:, s, :, :, h:], wt_ap[:, s, :, :, h:])
            else:
                nc.sync.dma_start(wt_all[:, s], wt_ap[:, s])

        # weights + stationary on the sync ring; x on the scalar ring so the
        # two streams transfer in parallel (one HWDGE ring ~250 GB/s).
        nc.sync.dma_start(xt_sb[:], xt_dram.ap().rearrange("g p i b -> p g i b"))
        for t in range(4):
            nc.scalar.dma_start(x_sb[:, t, :], x_r[:, t, :])
        for s in range(NSTRIPE):
            _wt_load(s)

        # rsqrt scratch
        magic = pp.tile([128, 16], I32)
        nc.gpsimd.memset(magic[:], 0x5F3759DF)
        rs_sh = pp.tile([128, 16], I32)
        rs_yi = pp.tile([128, 16], I32)
        rs_h = pp.tile([128, 16], F32)
        rs_t1 = pp.tile([128, 16], F32)
        rs_t2 = pp.tile([128, 16], F32)
        rs_tiles = (magic, rs_sh, rs_yi, rs_h, rs_t1, rs_t2)

        # per-sample 1/||x|| and the folded exp scales S/(XSCALE*WSCALE*||x||).
        # Each column runs standalone so scale_t unblocks ACT(b_t, s0) ASAP.
        xn2 = pp.tile([128, 4], F32)
        sq_dump = pp.tile([128, D], BF16)
        scales = pp.tile([128, 4], F32)
        rnorm = pp.tile([128, 4], F32)
        for t in range(4):
            nc.vector.scalar_tensor_tensor(
                out=sq_dump[:], in0=x_sb[:, t, :], scalar=1.0,
                in1=x_sb[:, t, :], op0=ALU.mult, op1=ALU.mult,
                accum_out=xn2[:, t : t + 1],
            )
            _emit_rsqrt(nc, rs_tiles, rnorm[:, t : t + 1], xn2[:, t : t + 1], 1)
            nc.vector.tensor_scalar(
                out=scales[:, t : t + 1], in0=rnorm[:, t : t + 1],
                scalar1=S / (XSCALE * WSCALE), scalar2=None, op0=ALU.mult,
            )

        sums = pp.tile([128, 4 * NSTRIPE + 1], F32)

        # ---------------- main loop: 7 stripes x 4 b_blocks ----
        for s in range(NSTRIPE):
            for t in range(4):
                ps = psum_pool.tile([128, STRIPE, 512], F32, tag="ps")
                for g in range(2):
                    lhsT = xt_sb[:, g, :, t * 128 : (t + 1) * 128]
                    for j in range(STRIPE):
                        nc.tensor.matmul(
                            ps[:, j, :CHUNK],
                            lhsT,
                            wt_all[:, s, g, :, j * CHUNK : (j + 1) * CHUNK],
                            start=(g == 0), stop=(g == 1),
                            perf_mode=DR,
                        )
                dump = pp.tile([128, STRIPE, CHUNK], BF16, tag=f"dump{t % 2}")
                if s == 0 and t == 0:
                    # split the very first activation: its first half only
                    # needs the first half-stripe DMA, starting ScalarE early
                    nc.scalar.activation(
                        dump[:, 0:2, :], ps[:, 0:2, :CHUNK], ACT.Exp,
                        scale=scales[:, 0:1],
                        accum_out=sums[:, 0:1],
                    )
                    nc.scalar.activation(
                        dump[:, 2:4, :], ps[:, 2:4, :CHUNK], ACT.Exp,
                        scale=scales[:, 0:1],
                        accum_out=sums[:, 4 * NSTRIPE : 4 * NSTRIPE + 1],
                    )
                else:
                    nc.scalar.activation(
                        dump[:], ps[:, :, :CHUNK], ACT.Exp,
                        scale=scales[:, t : t + 1],
                        accum_out=sums[:, t * NSTRIPE + s : t * NSTRIPE + s + 1],
                    )

        nc.scalar.dma_start(sums_dram.ap(), sums[:])

        # ---- phase 0b: label-side + margin math (off the critical path) ----
        from concourse.tile import add_dep_helper

        wl_sb = pp.tile([128, 4, D], BF16)
        nc.gpsimd.dma_start(
            wl_sb[:], wl_dram.ap().rearrange("(t p) d -> p t d", p=128)
        )
        nl2 = pp.tile([128, 4], F32)
        dotl = pp.tile([128, 4], F32)
        for t in range(4):
            nc.vector.scalar_tensor_tensor(
                out=sq_dump[:], in0=wl_sb[:, t, :], scalar=1.0,
                in1=wl_sb[:, t, :], op0=ALU.mult, op1=ALU.mult,
                accum_out=nl2[:, t : t + 1],
            )
        for t in range(4):
            nc.vector.scalar_tensor_tensor(
                out=sq_dump[:], in0=wl_sb[:, t, :], scalar=1.0,
                in1=x_sb[:, t, :], op0=ALU.mult, op1=ALU.mult,
                accum_out=dotl[:, t : t + 1],
            )
        xnorm = pp.tile([128, 4], F32)
        nc.vector.tensor_mul(xnorm[:], xn2[:], rnorm[:])
        rwl = pp.tile([128, 4], F32)
        _emit_rsqrt(nc, rs_tiles, rwl[:], nl2[:], 4)

        # margin params from clipped ||x||
        misc = pp.tile([128, 12], F32)
        xcl = pp.tile([128, 4], F32)
        nc.vector.tensor_scalar(
            out=xcl[:], in0=xnorm[:], scalar1=float(N_L), scalar2=float(N_U),
            op0=ALU.max, op1=ALU.min,
        )
        am = pp.tile([128, 4], F32)
        slope = (M_U - M_L) / (N_U - N_L)
        nc.vector.tensor_scalar(
            out=am[:], in0=xcl[:], scalar1=slope,
            scalar2=M_L - slope * N_L, op0=ALU.mult, op1=ALU.add,
        )
        # sin/cos of the margin angle via Taylor series on DVE (am in [0.1, 1])
        c2 = pp.tile([128, 4], F32)
        nc.vector.tensor_mul(c2[:], am[:], am[:])
        tser = pp.tile([128, 4], F32)
        sin_m = pp.tile([128, 4], F32)
        nc.vector.tensor_scalar(
            out=tser[:], in0=c2[:], scalar1=-1.0 / 72, scalar2=1.0,
            op0=ALU.mult, op1=ALU.add,
        )
        for dv in (42.0, 20.0, 6.0):
            nc.vector.tensor_mul(tser[:], tser[:], c2[:])
            nc.vector.tensor_scalar(
                out=tser[:], in0=tser[:], scalar1=-1.0 / dv, scalar2=1.0,
                op0=ALU.mult, op1=ALU.add,
            )
        nc.vector.tensor_mul(sin_m[:], tser[:], am[:])
        cos_m = pp.tile([128, 4], F32)
        nc.vector.tensor_scalar(
            out=tser[:], in0=c2[:], scalar1=-1.0 / 56, scalar2=1.0,
            op0=ALU.mult, op1=ALU.add,
        )
        for dv in (30.0, 12.0, 2.0):
            nc.vector.tensor_mul(tser[:], tser[:], c2[:])
            nc.vector.tensor_scalar(
                out=tser[:], in0=tser[:], scalar1=-1.0 / dv, scalar2=1.0,
                op0=ALU.mult, op1=ALU.add,
            )
        nc.vector.tensor_copy(cos_m[:], tser[:])
        mm_t = pp.tile([128, 4], F32)
        nc.vector.tensor_mul(mm_t[:], sin_m[:], am[:])
        thn = pp.tile([128, 4], F32)
        nc.vector.tensor_scalar(
            out=thn[:], in0=cos_m[:], scalar1=-1.0, scalar2=None, op0=ALU.mult
        )

        # loss_g = xcl/N_U^2 + 1/xcl  -> misc[:, 8:12]
        rxcl = pp.tile([128, 4], F32)
        nc.vector.reciprocal(rxcl[:], xcl[:])
        gl = pp.tile([128, 4], F32)
        nc.vector.tensor_scalar(
            out=gl[:], in0=xcl[:], scalar1=1.0 / (N_U * N_U), scalar2=None,
            op0=ALU.mult,
        )
        nc.vector.tensor_add(misc[:, 8:12], gl[:], rxcl[:])

        # cos_label -> misc[:, 4:8]
        cos_l = pp.tile([128, 4], F32)
        nc.vector.tensor_mul(cos_l[:], dotl[:], rwl[:])
        nc.vector.tensor_mul(cos_l[:], cos_l[:], rnorm[:])
        nc.vector.tensor_copy(misc[:, 4:8], cos_l[:])

        # sin_label = sqrt(1 - cos_l^2) via Newton rsqrt
        u = pp.tile([128, 4], F32)
        nc.vector.tensor_mul(u[:], cos_l[:], cos_l[:])
        nc.vector.tensor_scalar(
            out=u[:], in0=u[:], scalar1=-1.0, scalar2=1.0, op0=ALU.mult, op1=ALU.add
        )
        ru = pp.tile([128, 4], F32)
        _emit_rsqrt(nc, rs_tiles, ru[:], u[:], 4)
        sin_l = pp.tile([128, 4], F32)
        nc.vector.tensor_mul(sin_l[:], u[:], ru[:])

        # phi = cos_l*cos_m - sin_l*sin_m  (or cos_l - mm when cos_l <= -cos_m)
        phi_a = pp.tile([128, 4], F32)
        nc.vector.tensor_mul(phi_a[:], cos_l[:], cos_m[:])
        phi_b = pp.tile([128, 4], F32)
        nc.vector.tensor_mul(phi_b[:], sin_l[:], sin_m[:])
        phi = pp.tile([128, 4], F32)
        nc.vector.tensor_sub(phi[:], phi_a[:], phi_b[:])
        altv = pp.tile([128, 4], F32)
        nc.vector.tensor_sub(altv[:], cos_l[:], mm_t[:])
        maskc = pp.tile([128, 4], F32)
        nc.vector.tensor_tensor(out=maskc[:], in0=cos_l[:], in1=thn[:], op=ALU.is_gt)
        # blend: phif = altv + maskc * (phi - altv)
        dphi = pp.tile([128, 4], F32)
        nc.vector.tensor_sub(dphi[:], phi[:], altv[:])
        nc.vector.tensor_mul(dphi[:], dphi[:], maskc[:])
        nc.vector.tensor_add(misc[:, 0:4], altv[:], dphi[:])
        nc.scalar.dma_start(misc_dram.ap(), misc[:])


def _build(repeat=1):
    from concourse import bass, bacc, tile, mybir

    F32 = mybir.dt.float32
    FP8 = mybir.dt.float8e4
    BF16 = mybir.dt.bfloat16

    nc = bacc.Bacc("TRN2", target_bir_lowering=False, debug=False)

    tensors = {
        "x": nc.dram_tensor("x", [B, D], BF16, kind="ExternalInput"),
        "xt": nc.dram_tensor("xt", [2, 128, 2, B], FP8, kind="ExternalInput"),
        "wt": nc.dram_tensor("wt", [128, NSTRIPE, 2, 2, SCLS], FP8, kind="ExternalInput"),
        "wl": nc.dram_tensor("wl", [B, D], BF16, kind="ExternalInput"),
        "sums": nc.dram_tensor("sums", [128, 4 * NSTRIPE + 1], F32, kind="ExternalOutput"),
        "misc": nc.dram_tensor("misc", [128, 12], F32, kind="ExternalOutput"),
    }

    with tile.TileContext(nc) as tc:
        for _ in range(repeat):
            _emit_body(nc, tc, tensors, mybir, bass)

    nc.compile()
    return nc


class Runner:
    """Persistent jitted 8-core runner (inputs stay device-resident)."""

    def __init__(self, repeat=1):
        import jax
        from jax.sharding import Mesh, PartitionSpec, NamedSharding
        from jax.experimental.shard_map import shard_map
        from concourse import bass2jax, mybir

        self.jax = jax
        nc = _build(repeat)
        self.nc = nc
        bass2jax.install_neuronx_cc_hook()

        partition_name = (
            nc.partition_id_tensor.name if nc.partition_id_tensor else None
        )
        in_names, out_names, out_avals, zero_shapes = [], [], [], []
        for alloc in nc.m.functions[0].allocations:
            if not isinstance(alloc, mybir.MemoryLocationSet):
                continue
            name = alloc.memorylocations[0].name
            if alloc.kind == "ExternalInput":
                if name == partition_name:
                    continue
                in_names.append(name)
            elif alloc.kind == "ExternalOutput":
                shape = tuple(alloc.tensor_shape)
                dtype = mybir.dt.np(alloc.dtype)
                out_names.append(name)
                out_avals.append(jax.core.ShapedArray(shape, dtype))
                zero_shapes.append((shape, dtype))
        self.in_names = in_names
        self.out_names = out_names
        self.out_avals = out_avals
        self.zero_shapes = zero_shapes
        n_params = len(in_names)
        n_outs = len(out_names)
        all_in_names = in_names + out_names
        if partition_name is not None:
            all_in_names = all_in_names + [partition_name]

        def _body(*args):
            operands = list(args)
            if partition_name is not None:
                operands.append(bass2jax.partition_id_tensor())
            outs = bass2jax._bass_exec_p.bind(
                *operands,
                out_avals=tuple(out_avals),
                in_names=tuple(all_in_names),
                out_names=tuple(out_names),
                lowering_input_output_aliases=(),
                sim_require_finite=True,
                sim_require_nnan=True,
                nc=nc,
            )
            return tuple(outs)

        devices = jax.devices()[:NCORES]
        self.mesh = Mesh(np.asarray(devices), ("core",))
        in_specs = (PartitionSpec("core"),) * (n_params + n_outs)
        out_specs = (PartitionSpec("core"),) * n_outs
        self.sharding = NamedSharding(self.mesh, PartitionSpec("core"))
        self.fn = jax.jit(
            shard_map(
                _body, mesh=self.mesh, in_specs=in_specs, out_specs=out_specs,
                check_rep=False,
            ),
            donate_argnums=tuple(range(n_params, n_params + n_outs)),
            keep_unused=True,
        )

    def put_inputs(self, in_maps):
        jax = self.jax
        concat = [
            np.concatenate([np.asarray(m[name]) for m in in_maps], axis=0)
            for name in self.in_names
        ]
        return [jax.device_put(a, self.sharding) for a in concat]

    def zeros(self):
        jax = self.jax
        return [
            jax.device_put(np.zeros((NCORES * s[0], *s[1:]), d), self.sharding)
            for (s, d) in self.zero_shapes
        ]

    def run(self, in_dev):
        out = self.fn(*in_dev, *self.zeros())
        self.jax.block_until_ready(out)
        return out

    def results(self, out_arrs):
        res = []
        for c in range(NCORES):
            res.append(
                {
                    name: np.asarray(out_arrs[i]).reshape(
                        NCORES, *self.out_avals[i].shape
                    )[c]
                    for i, name in enumerate(self.out_names)
                }
            )
        return res


def _get_runner(repeat=1):
    key = ("runner", repeat)
    if key not in _cache:
        _cache[key] = Runner(repeat)
    return _cache[key]


def _make_in_maps(x, label, weight):
    import ml_dtypes

    FP8 = ml_dtypes.float8_e4m3

    x = np.asarray(x, dtype=np.float32)
    label = np.asarray(label)
    weight = np.asarray(weight, dtype=np.float32)
    wl = np.ascontiguousarray(weight[label]).astype(ml_dtypes.bfloat16)
    x_bf = x.astype(ml_dtypes.bfloat16)

    # xt: [2g, 128p, 2i, 512b] fp8 of XSCALE * x^T (d = g*256 + i*128 + p)
    xt = np.ascontiguousarray(
        (XSCALE * x).T.reshape(2, 2, 128, B).transpose(0, 2, 1, 3)
    ).astype(FP8)

    # normalized weight rows scaled by WSCALE, fp8, transposed per shard
    wn = weight * (WSCALE / np.linalg.norm(weight, axis=1, keepdims=True))
    in_maps = []
    for c in range(NCORES):
        shard = np.zeros((C_PAD, D), dtype=np.float32)
        shard[:C_SH] = wn[c * C_SH : (c + 1) * C_SH]
        wt = np.ascontiguousarray(
            shard.T.reshape(2, 2, 128, NSTRIPE, SCLS).transpose(2, 3, 0, 1, 4)
        ).astype(FP8)
        in_maps.append({"x": x_bf, "xt": xt, "wt": wt, "wl": wl})
    return in_maps


def _combine(results, fb_ctx=None):
    # per-core per-(sample, b_block, stripe) sum-exp
    raw29 = np.stack(
        [np.asarray(r["sums"], dtype=np.float64) for r in results]
    )  # [core, p, 29]: col 28 is the second half of (t=0, s=0)
    raw29[:, :, 0] += raw29[:, :, 4 * NSTRIPE]
    raw = raw29[:, :, : 4 * NSTRIPE].reshape(NCORES, 128, 4, NSTRIPE)
    misc = np.asarray(results[0]["misc"], dtype=np.float64)

    phi = misc[:, 0:4].T.reshape(B)
    cos_l = misc[:, 4:8].T.reshape(B)
    loss_g = misc[:, 8:12].T.reshape(B)

    # zero-pad classes (44 per core, all in the last stripe) contribute exp(0)=1
    raw[:, :, :, NSTRIPE - 1] -= NPADC
    sums_bs = raw.transpose(0, 2, 1, 3).reshape(NCORES, B, NSTRIPE)  # [core, b, s]

    sumexp_tot = sums_bs.sum(axis=(0, 2))
    corrected = sumexp_tot - np.exp(S * cos_l) + np.exp(S * phi)
    ce = np.log(corrected) - S * phi
    total = ce.mean() + LAMBDA_G * loss_g.mean()

    # top-1 accuracy via provable bounds on max_{c != label} cos_c:
    # for the max stripe-sum m (label term removed), ln(m)/S >= maxcos >=
    # ln(m/SCLS)/S. Samples falling between the bounds are resolved exactly
    # on the host (fb_ctx provides the raw inputs).
    b_idx = np.arange(B)
    lcore = None
    if fb_ctx is not None:
        _, label, _ = fb_ctx
        lcore = (np.asarray(label) // C_SH).astype(int)
        lstripe = ((np.asarray(label) % C_SH) // SCLS).astype(int)
        sums_nl = sums_bs.copy()
        sums_nl[lcore, b_idx, lstripe] -= np.exp(S * cos_l)
        sums_nl = np.maximum(sums_nl, 1e-300)
    else:
        sums_nl = np.maximum(sums_bs, 1e-300)
    mstripe = sums_nl.max(axis=(0, 2))
    ub = np.log(mstripe) / S
    lb = ub - math.log(SCLS) / S
    correct = phi > ub
    amb = (phi <= ub) & (phi >= lb)
    if amb.any() and fb_ctx is not None:
        x, label, weight = fb_ctx
        for b in np.nonzero(amb)[0]:
            xb = np.asarray(x[b], np.float64)
            cos_b = (weight @ xb) / (
                np.linalg.norm(weight, axis=1) * np.linalg.norm(xb)
            )
            cos_b[int(label[b])] = -2.0
            correct[b] = phi[b] > cos_b.max()
    prec1 = 100.0 * correct.mean()
    return np.float32(total), np.float32(prec1)


def kernel(x, label, weight):
    runner = _get_runner(1)
    in_dev = runner.put_inputs(_make_in_maps(x, label, weight))
    out = runner.run(in_dev)
    x = np.asarray(x, dtype=np.float32)
    weight = np.asarray(weight, dtype=np.float32)
    return _combine(runner.results(out), fb_ctx=(x, np.asarray(label), weight))


# revision 27
# speedup vs baseline: 1.0055x; 1.0055x over previous
"""Trainium2 kernel for MagFace/AdaCos-style margin softmax-CE loss.

Strategy (8 cores, class-parallel, fp8 DoubleRow):
  - Shard C=100000 classes across 8 cores (12500 each, zero-padded to
    12544 = 28 chunks of 448).
  - Host pre-normalizes weight rows (w_c/||w_c||, scaled by 64) and ships
    them fp8e4m3 in a transposed [d -> class] layout; x is shipped both
    raw fp32 (for norms + the label-side margin math) and as a 16x-scaled
    fp8 transposed copy (the matmul stationary operand).
  - Per core: matmul orientation is out[128 samples, 448 classes]:
    stationary lhsT = xT[128d, 2, 128b] fp8, moving rhs = wT[128d, 2, 448c]
    fp8 with perf_mode=DoubleRow (256-contraction per instr, 2 instr per
    chunk cover D=512). PSUM tiles [128, 4, 512] (4 banks), double-buffered.
  - ScalarE: one exp activation per (b_block, stripe-of-4-chunks) reading
    [128, 4, 448] from PSUM with per-partition scale S/(1024*||x_b||);
    accum_out yields the per-sample sum-exp directly. This is the pace
    setter (~1.92us per activation, 28 total).
  - Top-1 accuracy needs max_c cos_c only coarsely: the per-stripe sums
    bound it (ln(m)/S >= maxcos >= ln(m/1792)/S); ambiguous samples (none
    for this data, margin ~0.09) fall back to an exact host check.
  - The label-column margin math (phi) runs off the critical path from
    host-gathered label rows W[label].
  - Host combines per-core sums, corrects the label column, adds the
    MagFace regularizer, computes top-1 accuracy.
"""

import math
import sys

sys.path.insert(0, "/opt/trn_rl_repo")
sys.path.insert(0, "/opt/trn_rl_repo/concourse")

import numpy as np

# ---- problem constants ----
B = 512
D = 512
C = 100000
NCORES = 8
C_SH = C // NCORES          # 12500
CHUNK = 448
NCHUNK = 28                 # 28 * 448 = 12544
C_PAD = NCHUNK * CHUNK      # 12544
NPADC = C_PAD - C_SH        # 44 zero-pad classes per core
STRIPE = 4                  # chunks per activation group
NSTRIPE = NCHUNK // STRIPE  # 7
SCLS = STRIPE * CHUNK       # 1792 classes per stripe
S = 30.0
N_U = 110.0
N_L = 10.0
M_U = 1.0
M_L = 0.1
LAMBDA_G = 35.0
XSCALE = 16.0               # host scale folded into fp8 x
WSCALE = 64.0               # host scale folded into fp8 normalized weights

_cache = {}


def _emit_rsqrt(nc, pp_tiles, out, n2_ap, G, final_mul=1.0):
    """out = final_mul / sqrt(n2) via bit-trick seed + 2 Newton iterations."""
    import concourse.mybir as mybir

    ALU = mybir.AluOpType
    magic, sh, yi, h, t1, t2 = pp_tiles
    n2i = n2_ap.bitcast(mybir.dt.int32)
    nc.vector.tensor_scalar(
        out=sh[:, :G], in0=n2i, scalar1=1, scalar2=None,
        op0=ALU.logical_shift_right,
    )
    nc.vector.tensor_sub(yi[:, :G], magic[:, :G], sh[:, :G])
    y = yi[:, :G].bitcast(mybir.dt.float32)
    nc.vector.tensor_scalar(
        out=h[:, :G], in0=n2_ap, scalar1=0.5, scalar2=None, op0=ALU.mult
    )
    # iter 1
    nc.vector.tensor_mul(t1[:, :G], y, y)
    nc.vector.tensor_mul(t1[:, :G], t1[:, :G], h[:, :G])
    nc.vector.tensor_scalar(
        out=t2[:, :G], in0=t1[:, :G], scalar1=-1.0, scalar2=1.5,
        op0=ALU.mult, op1=ALU.add,
    )
    nc.vector.tensor_mul(t2[:, :G], t2[:, :G], y)
    # iter 2 (fold final_mul into the last step)
    nc.vector.tensor_mul(t1[:, :G], t2[:, :G], t2[:, :G])
    nc.vector.tensor_mul(t1[:, :G], t1[:, :G], h[:, :G])
    nc.vector.tensor_scalar(
        out=t1[:, :G], in0=t1[:, :G], scalar1=-final_mul, scalar2=1.5 * final_mul,
        op0=ALU.mult, op1=ALU.add,
    )
    nc.vector.tensor_mul(out, t1[:, :G], t2[:, :G])


def _emit_body(nc, tc, tensors, mybir, bass):
    F32 = mybir.dt.float32
    BF16 = mybir.dt.bfloat16
    FP8 = mybir.dt.float8e4
    I32 = mybir.dt.int32
    ALU = mybir.AluOpType
    ACT = mybir.ActivationFunctionType
    DR = mybir.MatmulPerfMode.DoubleRow

    x_dram = tensors["x"]
    xt_dram = tensors["xt"]
    wt_dram = tensors["wt"]
    wl_dram = tensors["wl"]
    sums_dram = tensors["sums"]
    misc_dram = tensors["misc"]
    wt_ap = wt_dram.ap()

    with (
        tc.tile_pool(name="persist", bufs=1) as pp,
        tc.tile_pool(name="psum", bufs=2, space=bass.MemorySpace.PSUM) as psum_pool,
    ):
        # ---- phase 0a: critical-path loads + exp table preload + PE warmup ----
        warm_src = pp.tile([128, 1], F32)
        nc.gpsimd.memset(warm_src[:], 0.0)
        warm_dst = pp.tile([128, 1], F32)
        nc.scalar.activation(warm_dst[:], warm_src[:], ACT.Exp)

        # PE warmup operands (memset, no DMA dependency): HAM unthrottles
        # during the DMA lead-in so the real matmul stream runs warm.
        warm_w = pp.tile([128, 2, 128], FP8)
        nc.gpsimd.memset(warm_w[:], 0)
        warm_m = pp.tile([128, 2, 512], FP8)
        nc.gpsimd.memset(warm_m[:], 0)
        warm_ps = psum_pool.tile([128, STRIPE, 512], F32, tag="ps")
        for _ in range(16):
            nc.tensor.matmul(
                warm_ps[:, 0, :], warm_w[:], warm_m[:],
                start=True, stop=True, perf_mode=DR,
            )

        # DMA order on the sync queue: x_t0 (gates scale0), xt (stationary),
        # wt stripe 0, then the rest of x interleaved with early stripes.
        x_sb = pp.tile([128, 4, D], BF16)
        x_r = x_dram.ap().rearrange("(t p) d -> p t d", p=128)
        xt_sb = pp.tile([128, 2, 2, B], FP8)
        wt_all = pp.tile([128, NSTRIPE, 2, 2, SCLS], FP8)

        def _wt_load(s):
            # all stripes load as halves: finer arrival granularity while the
            # PE prefetch runs ahead of the DMA stream (steady state has slack)
            h = SCLS // 2
            nc.sync.dma_start(wt_all[:, s, :, :, :h], wt_ap[:, s, :, :, :h])
            nc.sync.dma_start(wt_all[:, s, :, :, h:], wt_ap[:, s, :, :, h:])

        # weights + stationary on the sync ring; x on the scalar ring so the
        # two streams transfer in parallel (one HWDGE ring ~250 GB/s).
        nc.sync.dma_start(xt_sb[:], xt_dram.ap().rearrange("g p i b -> p g i b"))
        for t in range(4):
            nc.scalar.dma_start(x_sb[:, t, :], x_r[:, t, :])
        for s in range(NSTRIPE):
            _wt_load(s)

        # rsqrt scratch
        magic = pp.tile([128, 16], I32)
        nc.gpsimd.memset(magic[:], 0x5F3759DF)
        rs_sh = pp.tile([128, 16], I32)
        rs_yi = pp.tile([128, 16], I32)
        rs_h = pp.tile([128, 16], F32)
        rs_t1 = pp.tile([128, 16], F32)
        rs_t2 = pp.tile([128, 16], F32)
        rs_tiles = (magic, rs_sh, rs_yi, rs_h, rs_t1, rs_t2)

        # per-sample 1/||x|| and the folded exp scales S/(XSCALE*WSCALE*||x||).
        # Each column runs standalone so scale_t unblocks ACT(b_t, s0) ASAP.
        xn2 = pp.tile([128, 4], F32)
        sq_dump = pp.tile([128, D], BF16)
        scales = pp.tile([128, 4], F32)
        rnorm = pp.tile([128, 4], F32)
        for t in range(4):
            nc.vector.scalar_tensor_tensor(
                out=sq_dump[:], in0=x_sb[:, t, :], scalar=1.0,
                in1=x_sb[:, t, :], op0=ALU.mult, op1=ALU.mult,
                accum_out=xn2[:, t : t + 1],
            )
            _emit_rsqrt(nc, rs_tiles, rnorm[:, t : t + 1], xn2[:, t : t + 1], 1)
            nc.vector.tensor_scalar(
                out=scales[:, t : t + 1], in0=rnorm[:, t : t + 1],
                scalar1=S / (XSCALE * WSCALE), scalar2=None, op0=ALU.mult,
            )

        sums = pp.tile([128, 4 * NSTRIPE + 1], F32)

        # ---------------- main loop: 7 stripes x 4 b_blocks ----
        for s in range(NSTRIPE):
            for t in range(4):
                ps = psum_pool.tile([128, STRIPE, 512], F32, tag="ps")
                for g in range(2):
                    lhsT = xt_sb[:, g, :, t * 128 : (t + 1) * 128]
                    for j in range(STRIPE):
                        nc.tensor.matmul(
                            ps[:, j, :CHUNK],
                            lhsT,
                            wt_all[:, s, g, :, j * CHUNK : (j + 1) * CHUNK],
                            start=(g == 0), stop=(g == 1),
                            perf_mode=DR,
                        )
                dump = pp.tile([128, STRIPE, CHUNK], BF16, tag=f"dump{t % 2}")
                if s == 0 and t == 0:
                    # split the very first activation: its first half only
                    # needs the first half-stripe DMA, starting ScalarE early
                    nc.scalar.activation(
                        dump[:, 0:2, :], ps[:, 0:2, :CHUNK], ACT.Exp,
                        scale=scales[:, 0:1],
                        accum_out=sums[:, 0:1],
                    )
                    nc.scalar.activation(
                        dump[:, 2:4, :], ps[:, 2:4, :CHUNK], ACT.Exp,
                        scale=scales[:, 0:1],
                        accum_out=sums[:, 4 * NSTRIPE : 4 * NSTRIPE + 1],
                    )
                else:
                    nc.scalar.activation(
                        dump[:], ps[:, :, :CHUNK], ACT.Exp,
                        scale=scales[:, t : t + 1],
                        accum_out=sums[:, t * NSTRIPE + s : t * NSTRIPE + s + 1],
                    )

        nc.scalar.dma_start(sums_dram.ap(), sums[:])

        # ---- phase 0b: label-side + margin math (off the critical path) ----
        from concourse.tile import add_dep_helper

        wl_sb = pp.tile([128, 4, D], BF16)
        nc.gpsimd.dma_start(
            wl_sb[:], wl_dram.ap().rearrange("(t p) d -> p t d", p=128)
        )
        nl2 = pp.tile([128, 4], F32)
        dotl = pp.tile([128, 4], F32)
        for t in range(4):
            nc.vector.scalar_tensor_tensor(
                out=sq_dump[:], in0=wl_sb[:, t, :], scalar=1.0,
                in1=wl_sb[:, t, :], op0=ALU.mult, op1=ALU.mult,
                accum_out=nl2[:, t : t + 1],
            )
        for t in range(4):
            nc.vector.scalar_tensor_tensor(
                out=sq_dump[:], in0=wl_sb[:, t, :], scalar=1.0,
                in1=x_sb[:, t, :], op0=ALU.mult, op1=ALU.mult,
                accum_out=dotl[:, t : t + 1],
            )
        xnorm = pp.tile([128, 4], F32)
        nc.vector.tensor_mul(xnorm[:], xn2[:], rnorm[:])
        rwl = pp.tile([128, 4], F32)
        _emit_rsqrt(nc, rs_tiles, rwl[:], nl2[:], 4)

        # margin params from clipped ||x||
        misc = pp.tile([128, 12], F32)
        xcl = pp.tile([128, 4], F32)
        nc.vector.tensor_scalar(
            out=xcl[:], in0=xnorm[:], scalar1=float(N_L), scalar2=float(N_U),
            op0=ALU.max, op1=ALU.min,
        )
        am = pp.tile([128, 4], F32)
        slope = (M_U - M_L) / (N_U - N_L)
        nc.vector.tensor_scalar(
            out=am[:], in0=xcl[:], scalar1=slope,
            scalar2=M_L - slope * N_L, op0=ALU.mult, op1=ALU.add,
        )
        # sin/cos of the margin angle via Taylor series on DVE (am in [0.1, 1])
        c2 = pp.tile([128, 4], F32)
        nc.vector.tensor_mul(c2[:], am[:], am[:])
        tser = pp.tile([128, 4], F32)
        sin_m = pp.tile([128, 4], F32)
        nc.vector.tensor_scalar(
            out=tser[:], in0=c2[:], scalar1=-1.0 / 72, scalar2=1.0,
            op0=ALU.mult, op1=ALU.add,
        )
        for dv in (42.0, 20.0, 6.0):
            nc.vector.tensor_mul(tser[:], tser[:], c2[:])
            nc.vector.tensor_scalar(
                out=tser[:], in0=tser[:], scalar1=-1.0 / dv, scalar2=1.0,
                op0=ALU.mult, op1=ALU.add,
            )
        nc.vector.tensor_mul(sin_m[:], tser[:], am[:])
        cos_m = pp.tile([128, 4], F32)
        nc.vector.tensor_scalar(
            out=tser[:], in0=c2[:], scalar1=-1.0 / 56, scalar2=1.0,
            op0=ALU.mult, op1=ALU.add,
        )
        for dv in (30.0, 12.0, 2.0):
            nc.vector.tensor_mul(tser[:], tser[:], c2[:])
            nc.vector.tensor_scalar(
                out=tser[:], in0=tser[:], scalar1=-1.0 / dv, scalar2=1.0,
                op0=ALU.mult, op1=ALU.add,
            )
        nc.vector.tensor_copy(cos_m[:], tser[:])
        mm_t = pp.tile([128, 4], F32)
        nc.vector.tensor_mul(mm_t[:], sin_m[:], am[:])
        thn = pp.tile([128, 4], F32)
        nc.vector.tensor_scalar(
            out=thn[:], in0=cos_m[:], scalar1=-1.0, scalar2=None, op0=ALU.mult
        )

        # loss_g = xcl/N_U^2 + 1/xcl  -> misc[:, 8:12]
        rxcl = pp.tile([128, 4], F32)
        nc.vector.reciprocal(rxcl[:], xcl[:])
        gl = pp.tile([128, 4], F32)
        nc.vector.tensor_scalar(
            out=gl[:], in0=xcl[:], scalar1=1.0 / (N_U * N_U), scalar2=None,
            op0=ALU.mult,
        )
        nc.vector.tensor_add(misc[:, 8:12], gl[:], rxcl[:])

        # cos_label -> misc[:, 4:8]
        cos_l = pp.tile([128, 4], F32)
        nc.vector.tensor_mul(cos_l[:], dotl[:], rwl[:])
        nc.vector.tensor_mul(cos_l[:], cos_l[:], rnorm[:])
        nc.vector.tensor_copy(misc[:, 4:8], cos_l[:])

        # sin_label = sqrt(1 - cos_l^2) via Newton rsqrt
        u = pp.tile([128, 4], F32)
        nc.vector.tensor_mul(u[:], cos_l[:], cos_l[:])
        nc.vector.tensor_scalar(
            out=u[:], in0=u[:], scalar1=-1.0, scalar2=1.0, op0=ALU.mult, op1=ALU.add
        )
        ru = pp.tile([128, 4], F32)
        _emit_rsqrt(nc, rs_tiles, ru[:], u[:], 4)
        sin_l = pp.tile([128, 4], F32)
        nc.vector.tensor_mul(sin_l[:], u[:], ru[:])

        # phi = cos_l*cos_m - sin_l*sin_m  (or cos_l - mm when cos_l <= -cos_m)
        phi_a = pp.tile([128, 4], F32)
        nc.vector.tensor_mul(phi_a[:], cos_l[:], cos_m[:])
        phi_b = pp.tile([128, 4], F32)
        nc.vector.tensor_mul(phi_b[:], sin_l[:], sin_m[:])
        phi = pp.tile([128, 4], F32)
        nc.vector.tensor_sub(phi[:], phi_a[:], phi_b[:])
        altv = pp.tile([128, 4], F32)
        nc.vector.tensor_sub(altv[:], cos_l[:], mm_t[:])
        maskc = pp.tile([128, 4], F32)
        nc.vector.tensor_tensor(out=maskc[:], in0=cos_l[:], in1=thn[:], op=ALU.is_gt)
        # blend: phif = altv + maskc * (phi - altv)
        dphi = pp.tile([128, 4], F32)
        nc.vector.tensor_sub(dphi[:], phi[:], altv[:])
        nc.vector.tensor_mul(dphi[:], dphi[:], maskc[:])
        nc.vector.tensor_add(misc[:, 0:4], altv[:], dphi[:])
        nc.scalar.dma_start(misc_dram.ap(), misc[:])


def _build(repeat=1):
    from concourse import bass, bacc, tile, mybir

    F32 = mybir.dt.float32
    FP8 = mybir.dt.float8e4
    BF16 = mybir.dt.bfloat16

    nc = bacc.Bacc("TRN2", target_bir_lowering=False, debug=False)

    tensors = {
        "x": nc.dram_tensor("x", [B, D], BF16, kind="ExternalInput"),
        "xt": nc.dram_tensor("xt", [2, 128, 2, B], FP8, kind="ExternalInput"),
        "wt": nc.dram_tensor("wt", [128, NSTRIPE, 2, 2, SCLS], FP8, kind="ExternalInput"),
        "wl": nc.dram_tensor("wl", [B, D], BF16, kind="ExternalInput"),
        "sums": nc.dram_tensor("sums", [128, 4 * NSTRIPE + 1], F32, kind="ExternalOutput"),
        "misc": nc.dram_tensor("misc", [128, 12], F32, kind="ExternalOutput"),
    }

    with tile.TileContext(nc) as tc:
        for _ in range(repeat):
            _emit_body(nc, tc, tensors, mybir, bass)

    nc.compile()
    return nc


class Runner:
    """Persistent jitted 8-core runner (inputs stay device-resident)."""

    def __init__(self, repeat=1):
        import jax
        from jax.sharding import Mesh, PartitionSpec, NamedSharding
        from jax.experimental.shard_map import shard_map
        from concourse import bass2jax, mybir

        self.jax = jax
        nc = _build(repeat)
        self.nc = nc
        bass2jax.install_neuronx_cc_hook()

        partition_name = (
            nc.partition_id_tensor.name if nc.partition_id_tensor else None
        )
        in_names, out_names, out_avals, zero_shapes = [], [], [], []
        for alloc in nc.m.functions[0].allocations:
            if not isinstance(alloc, mybir.MemoryLocationSet):
                continue
            name = alloc.memorylocations[0].name
            if alloc.kind == "ExternalInput":
                if name == partition_name:
                    continue
                in_names.append(name)
            elif alloc.kind == "ExternalOutput":
                shape = tuple(alloc.tensor_shape)
                dtype = mybir.dt.np(alloc.dtype)
                out_names.append(name)
                out_avals.append(jax.core.ShapedArray(shape, dtype))
                zero_shapes.append((shape, dtype))
        self.in_names = in_names
        self.out_names = out_names
        self.out_avals = out_avals
        self.zero_shapes = zero_shapes
        n_params = len(in_names)
        n_outs = len(out_names)
        all_in_names = in_names + out_names
        if partition_name is not None:
            all_in_names = all_in_names + [partition_name]

        def _body(*args):
            operands = list(args)
            if partition_name is not None:
                operands.append(bass2jax.partition_id_tensor())
            outs = bass2jax._bass_exec_p.bind(
                *operands,
                out_avals=tuple(out_avals),
                in_names=tuple(all_in_names),
                out_names=tuple(out_names),
                lowering_input_output_aliases=(),
                sim_require_finite=True,
                sim_require_nnan=True,
                nc=nc,
            )
            return tuple(outs)

        devices = jax.devices()[:NCORES]
        self.mesh = Mesh(np.asarray(devices), ("core",))
        in_specs = (PartitionSpec("core"),) * (n_params + n_outs)
        out_specs = (PartitionSpec("core"),) * n_outs
        self.sharding = NamedSharding(self.mesh, PartitionSpec("core"))
        self.fn = jax.jit(
            shard_map(
                _body, mesh=self.mesh, in_specs=in_specs, out_specs=out_specs,
                check_rep=False,
            ),
            donate_argnums=tuple(range(n_params, n_params + n_outs)),
            keep_unused=True,
        )

    def put_inputs(self, in_maps):
        jax = self.jax
        concat = [
            np.concatenate([np.asarray(m[name]) for m in in_maps], axis=0)
            for name in self.in_names
        ]
        return [jax.device_put(a, self.sharding) for a in concat]

    def zeros(self):
        jax = self.jax
        return [
            jax.device_put(np.zeros((NCORES * s[0], *s[1:]), d), self.sharding)
            for (s, d) in self.zero_shapes
        ]

    def run(self, in_dev):
        out = self.fn(*in_dev, *self.zeros())
        self.jax.block_until_ready(out)
        return out

    def results(self, out_arrs):
        res = []
        for c in range(NCORES):
            res.append(
                {
                    name: np.asarray(out_arrs[i]).reshape(
                        NCORES, *self.out_avals[i].shape
                    )[c]
                    for i, name in enumerate(self.out_names)
                }
            )
        return res


def _get_runner(repeat=1):
    key = ("runner", repeat)
    if key not in _cache:
        _cache[key] = Runner(repeat)
    return _cache[key]


def _make_in_maps(x, label, weight):
    import ml_dtypes

    FP8 = ml_dtypes.float8_e4m3

    x = np.asarray(x, dtype=np.float32)
    label = np.asarray(label)
    weight = np.asarray(weight, dtype=np.float32)
    wl = np.ascontiguousarray(weight[label]).astype(ml_dtypes.bfloat16)
    x_bf = x.astype(ml_dtypes.bfloat16)

    # xt: [2g, 128p, 2i, 512b] fp8 of XSCALE * x^T (d = g*256 + i*128 + p)
    xt = np.ascontiguousarray(
        (XSCALE * x).T.reshape(2, 2, 128, B).transpose(0, 2, 1, 3)
    ).astype(FP8)

    # normalized weight rows scaled by WSCALE, fp8, transposed per shard
    wn = weight * (WSCALE / np.linalg.norm(weight, axis=1, keepdims=True))
    in_maps = []
    for c in range(NCORES):
        shard = np.zeros((C_PAD, D), dtype=np.float32)
        shard[:C_SH] = wn[c * C_SH : (c + 1) * C_SH]
        wt = np.ascontiguousarray(
            shard.T.reshape(2, 2, 128, NSTRIPE, SCLS).transpose(2, 3, 0, 1, 4)
        ).astype(FP8)
        in_maps.append({"x": x_bf, "xt": xt, "wt": wt, "wl": wl})
    return in_maps


def _combine(results, fb_ctx=None):
    # per-core per-(sample, b_block, stripe) sum-exp
    raw29 = np.stack(
        [np.asarray(r["sums"], dtype=np.float64) for r in results]
    )  # [core, p, 29]: col 28 is the second half of (t=0, s=0)
    raw29[:, :, 0] += raw29[:, :, 4 * NSTRIPE]
    raw = raw29[:, :, : 4 * NSTRIPE].reshape(NCORES, 128, 4, NSTRIPE)
    misc = np.asarray(results[0]["misc"], dtype=np.float64)

    phi = misc[:, 0:4].T.reshape(B)
    cos_l = misc[:, 4:8].T.reshape(B)
    loss_g = misc[:, 8:12].T.reshape(B)

    # zero-pad classes (44 per core, all in the last stripe) contribute exp(0)=1
    raw[:, :, :, NSTRIPE - 1] -= NPADC
    sums_bs = raw.transpose(0, 2, 1, 3).reshape(NCORES, B, NSTRIPE)  # [core, b, s]

    sumexp_tot = sums_bs.sum(axis=(0, 2))
    corrected = sumexp_tot - np.exp(S * cos_l) + np.exp(S * phi)
    ce = np.log(corrected) - S * phi
    total = ce.mean() + LAMBDA_G * loss_g.mean()

    # top-1 accuracy via provable bounds on max_{c != label} cos_c:
    # for the max stripe-sum m (label term removed), ln(m)/S >= maxcos >=
    # ln(m/SCLS)/S. Samples falling between the bounds are resolved exactly
    # on the host (fb_ctx provides the raw inputs).
    b_idx = np.arange(B)
    lcore = None
    if fb_ctx is not None:
        _, label, _ = fb_ctx
        lcore = (np.asarray(label) // C_SH).astype(int)
        lstripe = ((np.asarray(label) % C_SH) // SCLS).astype(int)
        sums_nl = sums_bs.copy()
        sums_nl[lcore, b_idx, lstripe] -= np.exp(S * cos_l)
        sums_nl = np.maximum(sums_nl, 1e-300)
    else:
        sums_nl = np.maximum(sums_bs, 1e-300)
    mstripe = sums_nl.max(axis=(0, 2))
    ub = np.log(mstripe) / S
    lb = ub - math.log(SCLS) / S
    correct = phi > ub
    amb = (phi <= ub) & (phi >= lb)
    if amb.any() and fb_ctx is not None:
        x, label, weight = fb_ctx
        for b in np.nonzero(amb)[0]:
            xb = np.asarray(x[b], np.float64)
            cos_b = (weight @ xb) / (
                np.linalg.norm(weight, axis=1) * np.linalg.norm(xb)
            )
            cos_b[int(label[b])] = -2.0
            correct[b] = phi[b] > cos_b.max()
    prec1 = 100.0 * correct.mean()
    return np.float32(total), np.float32(prec1)


def kernel(x, label, weight):
    runner = _get_runner(1)
    in_dev = runner.put_inputs(_make_in_maps(x, label, weight))
    out = runner.run(in_dev)
    x = np.asarray(x, dtype=np.float32)
    weight = np.asarray(weight, dtype=np.float32)
    return _combine(runner.results(out), fb_ctx=(x, np.asarray(label), weight))
